# revision 68
# baseline (speedup 1.0000x reference)
"""CAM (channel self-attention) kernel for Trainium2 — 8 NeuronCores, batch-parallel.

Math per batch element b (A = x[b] reshaped [N=4096, C=512]):
    G = A^T A                  [C, C]   (symmetric!)
    P = softmax_rows(G)        [C, C]
    Y = A P                    [N, C]
    out = gamma * Y + x

Sharding: data-parallel over batch — core i handles batch element i.
No cross-core communication needed.

Per-core schedule (engine-balanced against the ~23.4us HBM load window):
  - DMA x in 512 KiB groups -> A32 (f32, resident); DVE tensor_scalar
    casts A8 (fp8e4) directly.  Fine-grained groups keep chunk arrival
    smoother than the PE's consumption so it never idles long enough to
    trip the HAM clock throttle.
  - Per 128-row chunk: 4 PE transposes of A8 (A^T blocks -> one PSUM
    bank -> one ACT copy to AT8; ACT converts fp8 at full rate where
    DVE/GpSimd CAST hits a slow path).
  - Per PAIR of chunks: upper-triangle Gram matmuls in fp8 DoubleRow
    (256 contraction rows per matmul, 2 elem/cycle stream; free dims
    512/384/256/128 exploiting G's symmetry).
  - Lower triangle of G reconstructed with 6 PE transposes of the upper
    blocks; softmax rows pipelined behind them (row 0 reads its PSUM
    accumulator directly). gamma is folded into P's normalization, so
    the Y matmuls produce gamma*Y and the epilogue is one add.
  - A dense accumulating dummy-matmul train (plus matmuls chained onto
    the softmax's intermediates) keeps the PE busy across the softmax
    latency — a ~2-3us PE-idle gap here lets the free-running HAM MID
    window re-throttle the PE to 1.2 GHz for the start of the Y phase.
  - Y = A P via fp8 DoubleRow (lhsT = AT8 pair-slice, rhs = P8
    pair-slice), q-major in groups of 4 output chunks so the first
    matmuls only need P rows 0-1; 8 PSUM banks double-buffer 2 groups.
  - epilogue adds x per chunk: DVE tensor_tensor direct from PSUM for
    ~2/3 of chunks, ACT psum-drain + GpSimd add for the rest; staged
    bf16 (rel err ~2e-3 vs the 2e-2 gate) so the store stream is 4 MiB
    instead of 8 and splits into parallel-draining pieces.
"""

import numpy as np

import concourse.tile as tile
from concourse import bacc, mybir
from concourse.bass_utils import run_bass_kernel_spmd
from concourse.masks import make_identity

B = 8
H = 64
W = 64
C = 512
HW = H * W            # 4096 rows per batch element
NT = HW // 128        # 32 row chunks of 128
CT = C // 128         # 4 col chunks of 128
OGRP = 4              # row chunks per output Y/epilogue group (8 PSUM banks
                      # double-buffer 2 groups; OGRP=2 with 4 groups in
                      # flight measured ~1us slower)
F32 = mybir.dt.float32
BF16 = mybir.dt.bfloat16
FP8 = mybir.dt.float8e4
DR = mybir.MatmulPerfMode.DoubleRow

_CACHE = {}


def _emit(nc, tc, out, x, gamma):
    from contextlib import ExitStack

    with ExitStack() as ctx:
        big = ctx.enter_context(tc.tile_pool(name="big", bufs=1))
        small = ctx.enter_context(tc.tile_pool(name="small", bufs=1))
        stat = ctx.enter_context(tc.tile_pool(name="stat", bufs=4))
        ostage = ctx.enter_context(tc.tile_pool(name="ostage", bufs=8))
        # PSUM pools for phase 1 + softmax region; released before the Y
        # phase so all 8 banks are available for Y double-buffering.
        gps = tc.alloc_tile_pool(name="gps", bufs=1, space="PSUM")
        wps = tc.alloc_tile_pool(name="wps", bufs=3, space="PSUM")

        A32 = big.tile([128, NT, C], F32)     # x rows, n on partitions
        A8 = big.tile([128, NT, C], FP8)      # fp8e4 cast (Gram + transposes)
        # A^T, c on partitions, fp8e4; [t, ci, n-block] layout so the
        # per-chunk PSUM drain is contiguous and the Y lhsT pair-slice
        # [Ki, Ko=2, dim] has a 128 B ci-stride.
        AT8 = big.tile([128, NT, CT, 128], FP8)
        G32 = big.tile([128, CT, C], F32)     # full Gram matrix in SBUF
        E32 = big.tile([128, CT, C], F32)     # exp(G - rowmax)
        P8 = big.tile([128, CT, C], FP8)      # softmax(G) in fp8e4

        ident = small.tile([128, 128], FP8)
        make_identity(nc, ident[:])
        ident32 = small.tile([128, 128], F32)
        make_identity(nc, ident32[:])

        gB = small.tile([128, 1], F32)        # gamma broadcast to all partitions

        # PE warm-up: the HAM clock gate holds the PE at 1.2 GHz until it has
        # been busy ~3.4us. The PE is otherwise idle until the first input
        # chunk lands (~10.4us = barrier 6.6 + DMA latency ~2.9 + cast), so
        # burn exactly that window with dummy matmuls on a zeroed scratch
        # tile; real matmuls then start at 2.4 GHz. Crucially keep this SHORT:
        # cold FD-512 matmuls cost ~427ns each, and overshooting delays the
        # whole phase-1 pipeline (an earlier version with 30 of these pushed
        # phase 1 out by ~8us).
        warm_src = small.tile([128, C], BF16)
        nc.vector.memset(warm_src[:], 0.0)
        z32 = small.tile([128, 128], F32)     # zero weights for no-op matmuls
        nc.vector.memset(z32[:], 0.0)
        warm_ps = wps.tile([128, C], F32, name="warm", tag="w")
        for wi in range(12):
            nc.tensor.matmul(
                warm_ps[:], warm_src[:, 0:128], warm_src[:],
                start=(wi == 0), stop=(wi == 11),
            )

        # Upper-triangle Gram accumulators: G[mi-chunk, mi*128:].
        # g1 (384 cols) and g3 (128 cols) share one PSUM bank.
        g0 = gps.tile([128, C], F32, name="g0", tag="g0")
        g13 = gps.tile([128, C], F32, name="g13", tag="g13")
        g2 = gps.tile([128, C - 256], F32, name="g2", tag="g2")
        g_ps = [g0[:], g13[:, 0:384], g2[:], g13[:, 384:512]]

        # First loads chunk-granular so the PE can start early, then 1 MiB.
        # Last loads chunk-granular too: a 4-chunk group only fires its
        # completion semaphore once, so the final chunks' Gram matmuls would
        # otherwise all wait for the full 1 MiB group to land.
        # 2-chunk groups throughout: completion semaphores fire every 512 KiB
        # so chunk data becomes visible at a finer grain than the PE's
        # consumption rate — otherwise the PE periodically outruns the
        # stream, and a ~2us idle gap can trip the HAM throttle.
        load_groups = [1, 1] + [2] * ((NT - 4) // 2) + [1, 1]
        assert sum(load_groups) == NT
        k0 = 0
        for gi, gsz in enumerate(load_groups):
            r0 = k0 * 128
            r1 = (k0 + gsz) * 128
            nc.sync.dma_start(
                A32[:, k0:k0 + gsz, :],
                x[r0:r1, :].rearrange("(t p) c -> p t c", p=128),
            )
            if gi == 0:
                # gamma: tiny load on the ACT HWDGE ring, off the input path
                nc.scalar.dma_start(gB[:], gamma[:])
            for j in range(gsz):
                k = k0 + j
                # fp8 cast on DVE. Must be TENSOR_SCALAR (x1.0), not
                # tensor_copy: the CAST opcode hits a ~4x slow path for fp8
                # destinations.
                nc.vector.tensor_scalar_mul(A8[:, k, :], A32[:, k, :], 1.0)

                def do_transposes(k=k):
                    # A^T blocks of this chunk -> one PSUM bank -> one ACT
                    # copy (ACT casts fp8 at full speed, unlike DVE/GpSimd).
                    # The HW requires fp8 transpose-mode PSUM writes to use
                    # element step 2, so tp interleaves a dead byte/element.
                    tp = wps.tile([128, CT * 128, 2], FP8, name="tp", tag="w")
                    for ci in range(CT):
                        nc.tensor.transpose(
                            tp[:, ci * 128:(ci + 1) * 128, 0:1],
                            A8[:, k, ci * 128:(ci + 1) * 128],
                            ident[:],
                        )
                    nc.scalar.copy(
                        AT8[:, k, :, :],
                        tp[:, :, 0:1].rearrange(
                            "p (ci n) one -> p ci (n one)", ci=CT),
                    )

                def do_gram(k=k):
                    # Upper-triangle Gram matmuls, one set per chunk PAIR in
                    # fp8 DoubleRow: contracts 256 rows/matmul at 2
                    # elem/cycle, halving phase-1 PE stream time vs bf16 —
                    # the PE stays slower than the 358 GB/s input stream, so
                    # it never starves mid-phase (which used to trip the HAM
                    # throttle).
                    if k % 2 != 1:
                        return
                    pt = k // 2
                    for mi in range(CT):
                        nc.tensor.matmul(
                            g_ps[mi],
                            A8[:, 2 * pt:2 * pt + 2, mi * 128:(mi + 1) * 128],
                            A8[:, 2 * pt:2 * pt + 2, mi * 128:],
                            start=(pt == 0),
                            stop=(pt == NT // 2 - 1),
                            perf_mode=DR,
                            # g1/g3 share a bank; per-element has_written
                            # makes disjoint-region groups safe on HW
                            skip_group_check=(mi % 2 == 1),
                        )

                if k >= NT - 2:
                    # Final chunks: the Gram stop gates the whole softmax/Y
                    # pipeline, while these chunks' A^T transposes aren't
                    # needed until their own Y matmuls much later — emit the
                    # Gram first so the PE retires it first.
                    do_gram()
                    do_transposes()
                else:
                    do_transposes()
                    do_gram()
            k0 += gsz

        # Post-Gram region, pipelined per row-chunk so the softmax of row 0
        # (whose full row lives in g0 — no lower-triangle blocks needed)
        # starts as soon as the Gram stops, while the PE fills the remaining
        # rows' lower-triangle blocks via transposes of the upper ones
        # (G[mi, j] = G[j, mi]^T for j < mi).
        def copy_upper(mi):
            if mi % 2 == 0:
                nc.vector.tensor_copy(G32[:, mi, mi * 128:], g_ps[mi])
            else:
                nc.scalar.copy(G32[:, mi, mi * 128:], g_ps[mi])

        def fill_lower(mi):
            for j in range(mi):
                lb = wps.tile([128, 128], F32, name="lb", tag="w")
                nc.tensor.transpose(
                    lb[:], G32[:, j, mi * 128:(mi + 1) * 128], ident32[:])
                if (mi + j) % 2 == 0:
                    nc.vector.tensor_copy(G32[:, mi, j * 128:(j + 1) * 128], lb[:])
                else:
                    nc.scalar.copy(G32[:, mi, j * 128:(j + 1) * 128], lb[:])

        def softmax_row(mi):
            # Row 0's full row is g0 (all-upper): read PSUM directly, off the
            # copy's critical path. Other rows need the assembled G32 row.
            src = g_ps[0] if mi == 0 else G32[:, mi, :]
            nmax = stat.tile([128, 1], F32)
            nc.vector.tensor_reduce(
                nmax[:], src,
                axis=mybir.AxisListType.X, op=mybir.AluOpType.max, negate=True,
            )
            esum = stat.tile([128, 1], F32)
            nc.scalar.activation(
                E32[:, mi, :], src,
                mybir.ActivationFunctionType.Exp,
                bias=nmax[:], scale=1.0, accum_out=esum[:],
            )
            rsum = stat.tile([128, 1], F32)
            nc.vector.reciprocal(rsum[:], esum[:])
            # Fold gamma into P: P8 = (gamma/rowsum) * E. The Y matmuls then
            # produce gamma*Y directly and the epilogue is a plain add.
            rsum_g = stat.tile([128, 1], F32)
            nc.vector.tensor_tensor(
                rsum_g[:], rsum[:], gB[:], op=mybir.AluOpType.mult)
            nc.vector.tensor_scalar_mul(P8[:, mi, :], E32[:, mi, :], rsum_g[:])

        # A PE-idle gap of even ~2us here can cross the free-running HAM
        # MID window and re-throttle the PE to 1.2 GHz for the start of
        # the Y phase (costing ~3us). Keep the PE visibly busy with
        # dummy matmuls: a dense unconditional train right after the
        # Gram stops, then matmuls dependency-chained onto the softmax
        # pipeline's intermediate products. These must come AFTER all
        # the lower-triangle transposes in PE program order, else they
        # serialize the transpose schedule against the softmax rows.
        # All accumulate into ONE PSUM tile (start/stop bracketed by the
        # caller) — per-ka tiles would serialize on the pool ring's WAR
        # semaphores and leave gaps in the train.
        ka_ps = [None]

        def keepalive(dep_lhsT, rhs, start=False, stop=False):
            if ka_ps[0] is None:
                ka_ps[0] = wps.tile([128, C], F32, name="ka", tag="w")
            nc.tensor.matmul(ka_ps[0][:, 0:rhs.free_size()], dep_lhsT, rhs,
                             start=start, stop=stop,
                             skip_group_check=True)

        # Interleave emission per row so each engine's queue orders row
        # mi's softmax ops BEFORE row mi+1's lower-triangle copies — with
        # all transposes emitted first, ACT would sit on 6 lb copies before
        # reaching exp0 and the rows would serialize.
        for mi in range(CT):
            copy_upper(mi)
        softmax_row(0)
        for mi in range(1, CT):
            fill_lower(mi)
            if mi < CT - 1:
                softmax_row(mi)
        # All PE transposes are emitted; now the keepalive train, then the
        # last softmax row. The train is sized to keep the PE busy through
        # the ENTIRE softmax latency (~3.5us): intermittent 200ns blips are
        # not enough — the free-running HAM window can still sample an idle
        # period and re-throttle the Y phase to 1.2 GHz.
        # Keepalive sizing matters both ways: sparse blips let the
        # free-running HAM window sample "idle" and re-throttle the Y phase
        # to 1.2 GHz (observed tripping at ~50% PE duty), while an oversized
        # train overruns the softmax's dependency window and delays Y. Six
        # unconditional matmuls bridge to the first exp; the exp-chained
        # keepalives use fp32 operands on purpose — a 4-pass fp32 FD-512
        # matmul runs ~850ns, long enough to cover a whole softmax pipeline
        # stage even when the chip is downclocked, with only 2 instructions
        # of overrun exposure.
        for i in range(10):
            keepalive(warm_src[:, 0:128], warm_src[:], start=(i == 0))
        keepalive(E32[:, 0, 0:128], E32[:, 0, :])
        keepalive(P8[:, 0, 0:128], P8[:, 0, :])
        keepalive(E32[:, 1, 0:128], E32[:, 1, :])
        keepalive(P8[:, 1, 0:128], P8[:, 1, :])
        keepalive(P8[:, 1, 0:128], P8[:, 1, :], stop=True)
        softmax_row(CT - 1)

        # Free the 6 phase-1 PSUM banks (LIFO order); Y takes all 8.
        wps.release()
        gps.release()
        yps = tc.alloc_tile_pool(name="yps", bufs=2 * OGRP, space="PSUM")

        # Y = A @ P, epilogue out = gamma * Y + x, staged bf16.
        # The output round-trips at bf16 precision (rel err ~2e-3, well
        # inside the 2e-2 gate); halving the store bytes takes the output
        # DMA off the phase-2 critical path.
        out_groups = [OGRP] * (NT // OGRP - 1) + [2, 1, 1]
        assert sum(out_groups) == NT
        t0 = 0
        for h, osz in enumerate(out_groups):
            r0 = t0 * 128
            r1 = (t0 + osz) * 128
            o16 = ostage.tile([128, OGRP, C], BF16, name="o16", tag="o16")
            ys = [yps.tile([128, C], F32, name="y", tag="y")
                  for _ in range(osz)]
            # q-major within the group: the q=0 matmuls only need P8 rows
            # 0-1, so they run while rows 2-3 are still in the softmax pipe.
            for q in range(CT // 2):
                if h == 0 and q == 1:
                    # The first group's q0->q1 join waits ~0.6us for P rows
                    # 2-3 — long enough for the free-running HAM window to
                    # re-throttle the PE. Fill it with a zero-weight fp32
                    # matmul (4-pass, ~850ns, adds exactly 0.0 to the
                    # accumulator) chained to exp row 2.
                    nc.tensor.matmul(
                        ys[0][:], z32[:], E32[:, 2, :],
                        start=False, stop=False, skip_group_check=True,
                    )
                for j in range(osz):
                    t = t0 + j
                    # fp8 DoubleRow: each matmul contracts 256 c-rows (2
                    # k-tiles packed 2/cell) and streams P at 2 elem/cycle.
                    # AT8/P8 are [128, CT, *]; slicing 2 adjacent ci gives
                    # the [Ki, Ko=2, dim] layout DoubleRow expects.
                    nc.tensor.matmul(
                        ys[j][:],
                        AT8[:, t, 2 * q:2 * q + 2, :],
                        P8[:, 2 * q:2 * q + 2, :],
                        start=(q == 0),
                        stop=(q == CT // 2 - 1),
                        perf_mode=DR,
                    )
            # Epilogue out = (gamma*Y) + x, with gamma pre-folded into P so
            # each chunk is a single elementwise add. Balance DVE (~0.6us,
            # direct from PSUM) against GpSimd (~1.2us, via a fast ACT
            # PSUM-drain since GpSimd has no PSUM port); mixed-dtype adds
            # from SBUF measured slower than PSUM-direct, so DVE reads the
            # Y banks itself.
            for j in range(osz):
                t = t0 + j
                # Tail groups stay on DVE: GpSimd's queue drains ~1.2us per
                # add, and a tail chunk queued behind its backlog stalls the
                # final store by several us. j==1 of each full group rides
                # GpSimd; the 4th add of alternate groups is column-split
                # DVE/GpSimd (320/192) — giving GpSimd a second full add put
                # those groups ~0.6us over the PE's 1.73us pace and stalled
                # bank recycling every other group.
                if osz == OGRP and j == 1:
                    y16 = ostage.tile([128, C], BF16, name="y16", tag="y16")
                    nc.scalar.copy(y16[:], ys[j][:])
                    nc.gpsimd.tensor_tensor(
                        o16[:, j, :], y16[:], A32[:, t, :],
                        op=mybir.AluOpType.add,
                    )
                elif osz == OGRP and j == 3 and h % 2 == 0:
                    y16 = ostage.tile([128, C], BF16, name="y16", tag="y16")
                    nc.scalar.copy(y16[:], ys[j][:])
                    nc.vector.tensor_tensor(
                        o16[:, j, 0:320], y16[:, 0:320], A32[:, t, 0:320],
                        op=mybir.AluOpType.add,
                    )
                    nc.gpsimd.tensor_tensor(
                        o16[:, j, 320:512], y16[:, 320:512],
                        A32[:, t, 320:512], op=mybir.AluOpType.add,
                    )
                else:
                    nc.vector.tensor_tensor(
                        o16[:, j, :], ys[j][:], A32[:, t, :],
                        op=mybir.AluOpType.add,
                    )
            # Stores. A single DMA instruction's packets (one 1 KiB row per
            # partition) drain through few engines (~45 GB/s); issuing more,
            # smaller stores promptly keeps several in flight so the HW
            # spreads them across DMA engines. The final single-chunk groups
            # are further split by partition halves onto two rings so the
            # very last drain runs in parallel.
            if osz > 1:
                # All on the Sync ring (idle in phase 2 — the scalar ring
                # would queue stores behind the ACT epilogue drains).
                for s0 in range(0, osz, 2):
                    s1 = min(s0 + 2, osz)
                    nc.sync.dma_start(
                        out[r0 + s0 * 128:r0 + s1 * 128, :]
                        .rearrange("(t p) c -> p t c", p=128),
                        o16[:, s0:s1, :],
                    )
            else:
                # final chunks: split by partition halves across two rings
                # so the very last drain parallelizes over DMA engines.
                # (A 3-way split adding the scalar ring measured a 12us
                # drain — keep to the sync+gpsimd pair.)
                half = out[r0:r1, :].rearrange("(t p) c -> p t c", p=128)
                nc.sync.dma_start(half[0:64], o16[0:64, 0:1, :])
                nc.gpsimd.dma_start(half[64:128], o16[64:128, 0:1, :])
            t0 += osz
        yps.release()


def build():
    nc = bacc.Bacc("TRN2", target_bir_lowering=False, debug=False)
    x = nc.dram_tensor("x", [HW, C], F32, kind="ExternalInput").ap()
    gamma = nc.dram_tensor("gamma", [128, 1], F32, kind="ExternalInput").ap()
    out = nc.dram_tensor("out", [HW, C], BF16, kind="ExternalOutput").ap()
    with tile.TileContext(nc) as tc:
        _emit(nc, tc, out, x, gamma)
    nc.compile()
    return nc


def kernel(x: np.ndarray, gamma: np.ndarray, trace: bool = False):
    assert x.shape == (B, H, W, C), x.shape
    if "nc" not in _CACHE:
        _CACHE["nc"] = build()
    nc = _CACHE["nc"]

    g128 = np.full((128, 1), np.float32(np.asarray(gamma).reshape(-1)[0]),
                   dtype=np.float32)
    in_maps = [
        {
            "x": np.ascontiguousarray(
                np.asarray(x[i], dtype=np.float32).reshape(HW, C)),
            "gamma": g128,
        }
        for i in range(B)
    ]
    if trace:
        res = run_bass_kernel_spmd(nc, in_maps, core_ids=list(range(B)),
                                   trace=True)
    else:
        # Force-untraced: a stray BASS_TRACE in the environment would route
        # through profiling hooks this image may not have.
        import os
        prev = os.environ.get("BASS_NEVER_TRACE")
        os.environ["BASS_NEVER_TRACE"] = "1"
        try:
            res = run_bass_kernel_spmd(nc, in_maps, core_ids=list(range(B)))
        finally:
            if prev is None:
                os.environ.pop("BASS_NEVER_TRACE", None)
            else:
                os.environ["BASS_NEVER_TRACE"] = prev
    _CACHE["last_result"] = res
    out = np.stack(
        [np.asarray(res.results[i]["out"], dtype=np.float32) for i in range(B)],
        axis=0,
    )
    return out.reshape(B, H, W, C)
